# revision 21
# baseline (speedup 1.0000x reference)
"""MoE layer (dense experts) on 8 Trainium2 NeuronCores via Bass/Tile.

Problem (hardcoded shapes):
  x        [4, 2048, 1024] f32
  gate_w   [1024, 8] f32, gate_b [8] f32
  expert_w [8, 1024, 1024] f32, expert_b [8, 1024] f32
  out[b,t,p] = sum_e softmax(x @ gate_w + gate_b)[b,t,e]
               * (x @ expert_w[e] + expert_b[e])[b,t,p]

Sharding: data-parallel over tokens. 8192 tokens are split into 8 shards of
1024; every core gets the full gate/expert weights (replicated) and computes
its token shard end-to-end. No collectives.

Per-core kernel (x pre-transposed on host so the contraction dim is the
partition dim for both matmul operands). Design notes from trace analysis:
steady state is already at the PE roofline (one N=512 bf16 matmul issued
every ~216ns, LDWEIGHTS hidden by the background weight buffer), so the
whole game is the ramp-in, the gating overhead, and the tail:

  - DMA order: gw first, then (xt_d, w0_d) pairs per contraction tile so
    expert-0 compute starts as soon as the first 512KB lands, then w1..w7
    (paced by the weight pool slots).
  - Warm-up matmuls on a memset tile fill the ~10us preamble+DMA-latency
    window so the PE's HAM clock gate is already at 2.4GHz when real work
    arrives.
  - Phase A (d-outer over 7 psum banks: ti0-2 x both p-halves + ti3 pc0)
    tracks the arriving DMA pairs 1:1. All gating logits for ti0-3 ride in
    one extra psum bank as a single accumulation group (one start=True mm
    clears the bank; later mms overwrite-or-accumulate per has_written),
    each issued right after an expert mm with the same stationary xt tile
    so its LDWEIGHTS hides under the 216ns expert matmul.
  - Phase B: remaining expert-0 groups group-major; gating for ti4-7 rides
    in the (ti4-7, pc0) groups the same way into a second logits bank.
  - Softmax: g = exp(logits) on ACT (no max subtraction: logits are O(6)
    here), then one reduce + reciprocal + 4 tiny scalar-muls per logits
    bank normalizes in place, long before the weights are consumed. The
    epilogue accumulates acc += g_e * psum_e on DVE; the last expert's
    epilogue writes the bf16 store tile directly. This removes all
    per-token-tile PE transposes, the gate-bias matmuls, and the
    expert-bias matmuls (biases are all zero in this problem; nonzero
    biases take a build-time fallback path).
  - Experts 1..7: plain group-major d-loops, epilogue chains on DVE.
"""

import os
from contextlib import ExitStack

import numpy as np

import concourse.bacc as bacc
import concourse.bass as bass
import concourse.mybir as mybir
import concourse.tile as tile
from concourse.bass_utils import run_bass_kernel_spmd

B, T, D, E, P = 4, 2048, 1024, 8, 1024
N_CORES = 8
TOK = B * T                # 8192 tokens
TS = TOK // N_CORES        # 1024 tokens per core
DT = D // 128              # 8 contraction tiles
TT = TS // 128             # 8 token tiles per core
PCHUNK = 512               # psum bank free size (f32)
PC = P // PCHUNK           # 2 p-chunks
WARM = int(os.environ.get("MOE_WARM", "26"))  # HAM warm-up matmuls (N=128)
OUT32 = os.environ.get("MOE_OUT32", "0") == "1"  # f32 output stores

_F32 = mybir.dt.float32
_BF16 = mybir.dt.bfloat16

TRACE = os.environ.get("MOE_TRACE", "0") == "1"  # test.py sets this

_build_cache = {}


def _build(has_gb: bool, has_eb: bool) -> bass.Bass:
    nc = bacc.Bacc("TRN2", target_bir_lowering=False, debug=False,
                   num_devices=N_CORES)

    xT = nc.dram_tensor("xT", [D, TS], _BF16, kind="ExternalInput").ap()
    # gw packed on host: gw_p[dp, dt*8+e] = gate_w[dt*128+dp, e]
    gw = nc.dram_tensor("gw_p", [128, DT * E], _BF16, kind="ExternalInput").ap()
    ew = nc.dram_tensor("expert_w", [E, D, P], _BF16, kind="ExternalInput").ap()
    if has_gb:
        gb4 = nc.dram_tensor("gb4", [1, 4 * E], _BF16, kind="ExternalInput").ap()
        ones = nc.dram_tensor("ones", [1, 128], _BF16, kind="ExternalInput").ap()
    if has_eb:
        eb = nc.dram_tensor("expert_b", [E, P], _BF16, kind="ExternalInput").ap()
        ident = nc.dram_tensor("ident", [128, 128], _F32,
                               kind="ExternalInput").ap()
    out_dt = _F32 if OUT32 else _BF16
    out = nc.dram_tensor("out", [TS, P], out_dt, kind="ExternalOutput").ap()

    out_t = out.rearrange("(tt tp) p -> tp tt p", tp=128)
    xT_t = xT.rearrange("(dt dp) t -> dp dt t", dp=128)

    with tile.TileContext(nc) as tc, ExitStack() as ctx:
        consts = ctx.enter_context(tc.tile_pool(name="consts", bufs=1))
        w_pool = ctx.enter_context(tc.tile_pool(name="w", bufs=32))
        stage_pool = ctx.enter_context(tc.tile_pool(name="stage", bufs=4))
        psum = ctx.enter_context(tc.tile_pool(name="psum", bufs=8, space="PSUM"))

        # --- warm-up: keep PE busy through the preamble + DMA latency so the
        # HAM clock gate is at 2.4GHz when the first real tiles land. The
        # memset rides on GpSimd (whose queue is free right at preamble end,
        # unlike DVE) and is narrow so the first matmul issues ASAP; N=128
        # matmuls give fine-grained filler that ends just before the first
        # (xt, w0) pair lands.
        wu = consts.tile([128, 128], _BF16, name="wu")
        nc.gpsimd.memset(wu[:, :], 1.0)
        wu_ps = psum.tile([128, PCHUNK], _F32, name="wu_ps", tag="ps")
        for _ in range(WARM):
            nc.tensor.matmul(wu_ps[:, :128], wu[:, :], wu[:, :],
                             start=True, stop=True)

        # --- DMA order: first (xt_0, w0_0) pair so expert-0 compute can
        # start ASAP, then gw (needed by the first gating mm, which only
        # issues after the pair-0 expert mms), then the remaining pairs,
        # then the other experts' weights (paced by the pool slots).
        gw_sb = consts.tile([128, DT, E], _BF16, name="gw_sb")
        xt = consts.tile([128, DT, TS], _BF16, name="xt")
        w0 = []
        HTS = TS // 2
        for di in range(DT):
            # Phase A only touches token columns 0..511 (ti0-3): ship that
            # half with w0 so the per-d critical DMA is 384KB, not 512KB —
            # phase A then runs PE-bound with no DMA-jitter stalls. The
            # ti4-7 halves (needed first by phase B) follow after the pairs.
            nc.sync.dma_start(xt[:, di, 0:HTS], xT_t[:, di, 0:HTS])
            w_tile = w_pool.tile([128, P], _BF16, name=f"wt0_{di}", tag="wt")
            nc.sync.dma_start(w_tile[:, :], ew[0, di * 128:(di + 1) * 128, :])
            w0.append(w_tile)
            if di == 0:
                nc.sync.dma_start(gw_sb[:, :, :],
                                  gw.rearrange("p (dt e) -> p dt e", dt=DT))
                if has_gb:
                    gb4_sb = consts.tile([1, 4 * E], _BF16, name="gb4_sb")
                    nc.sync.dma_start(gb4_sb[:, :], gb4)
                    ones_sb = consts.tile([1, 128], _BF16, name="ones_sb")
                    nc.sync.dma_start(ones_sb[:, :], ones)
                if has_eb:
                    eb_sb = consts.tile([E, P], _BF16, name="eb_sb")
                    nc.sync.dma_start(eb_sb[:, :], eb)
                    id_sb = consts.tile([128, 128], _F32, name="id_sb")
                    nc.sync.dma_start(id_sb[:, :], ident)
        for di in range(DT):
            nc.sync.dma_start(xt[:, di, HTS:TS], xT_t[:, di, HTS:TS])

        ug_sb = consts.tile([128, TT, E], _F32, name="ug_sb")   # exp(logits)
        esum = consts.tile([128, TT], _F32, name="esum")
        rec = consts.tile([128, TT], _F32, name="rec")
        acc = consts.tile([128, TT, P], _F32, name="acc")
        if has_eb:
            ugt_sb = consts.tile([E, TS], _BF16, name="ugt_sb")

        def gate_mm(lg, ti, lo, di, start, stop):
            # logits[t, e] for token tile ti into the packed logits bank;
            # shares its stationary xt tile with the expert mm just issued.
            nc.tensor.matmul(lg[:, (ti - lo) * E:(ti - lo + 1) * E],
                             xt[:, di, ti * 128:(ti + 1) * 128],
                             gw_sb[:, di, :], start=start, stop=stop)

        def expert_mm(ps, e_wt, ti, pc, di, start, stop):
            nc.tensor.matmul(ps[:, :],
                             xt[:, di, ti * 128:(ti + 1) * 128],
                             e_wt[di][:, pc * PCHUNK:(pc + 1) * PCHUNK],
                             start=start, stop=stop)

        def epilogue(e, ti, pc, ps):
            ug_col = ug_sb[:, ti, e:e + 1]
            acc_sl = acc[:, ti, pc * PCHUNK:(pc + 1) * PCHUNK]
            if e == 0:
                nc.vector.tensor_scalar_mul(acc_sl, ps[:, :], ug_col)
            elif e < E - 1:
                nc.vector.scalar_tensor_tensor(
                    acc_sl, ps[:, :], ug_col, acc_sl,
                    op0=mybir.AluOpType.mult, op1=mybir.AluOpType.add)
            else:
                stg = stage_pool.tile([128, PCHUNK], out_dt, name="stg")
                if has_eb:
                    # gate-weighted expert_b mix: gT.T @ eb, K=E
                    ps_b = psum.tile([128, PCHUNK], _F32,
                                     name=f"psb{ti}_{pc}", tag="ps")
                    nc.tensor.matmul(
                        ps_b[:, :], ugt_sb[:, ti * 128:(ti + 1) * 128],
                        eb_sb[:, pc * PCHUNK:(pc + 1) * PCHUNK],
                        start=True, stop=True)
                    tmp = stage_pool.tile([128, PCHUNK], _F32, name="tmp",
                                          tag="tmp")
                    nc.vector.scalar_tensor_tensor(
                        tmp[:, :], ps[:, :], ug_col, acc_sl,
                        op0=mybir.AluOpType.mult, op1=mybir.AluOpType.add)
                    nc.vector.tensor_tensor(
                        stg[:, :], tmp[:, :], ps_b[:, :],
                        op=mybir.AluOpType.add)
                else:
                    nc.vector.scalar_tensor_tensor(
                        stg[:, :], ps[:, :], ug_col, acc_sl,
                        op0=mybir.AluOpType.mult, op1=mybir.AluOpType.add)
                nc.sync.dma_start(
                    out_t[:, ti, pc * PCHUNK:(pc + 1) * PCHUNK], stg[:, :])

        # ---------------- expert 0 + gating ----------------
        # Phase A: d-outer over 7 banks (ti0-2 x pc0/pc1, ti3 pc0) + the
        # ti0-3 logits bank; tracks the (xt_d, w0_d) DMA pairs.
        lgA = psum.tile([128, 4 * E], _F32, name="lgA", tag="ps")
        ps_a = {}
        for ti in range(3):
            for pc in range(PC):
                ps_a[ti, pc] = psum.tile([128, PCHUNK], _F32,
                                         name=f"psA{ti}_{pc}", tag="ps")
        ps_a[3, 0] = psum.tile([128, PCHUNK], _F32, name="psA3_0", tag="ps")

        if has_gb:
            # rank-1 gate-bias preload as lgA's group opener
            nc.tensor.matmul(lgA[:, :], ones_sb[:1, :], gb4_sb[:1, :],
                             start=True, stop=False)
        for di in range(DT):
            last = di == DT - 1
            if last:
                # ti3 first on the last d-tile so lgA's group closes a few
                # matmuls early — its bank (and exp/normalize) then frees
                # right as phase A ends, un-stalling phase B's first group.
                expert_mm(ps_a[3, 0], w0, 3, 0, di, False, True)
                gate_mm(lgA, 3, 0, di, start=False, stop=False)
            for ti in range(3):
                expert_mm(ps_a[ti, 0], w0, ti, 0, di, di == 0, last)
                expert_mm(ps_a[ti, 1], w0, ti, 1, di, di == 0, last)
                gate_mm(lgA, ti, 0, di,
                        start=(di == 0 and ti == 0 and not has_gb),
                        stop=(last and ti == 2))
            if not last:
                expert_mm(ps_a[3, 0], w0, 3, 0, di, di == 0, False)
                gate_mm(lgA, 3, 0, di, start=False, stop=False)

        def normalize(lo):
            # g = exp(logits) / rowsum, in place on ug_sb[:, lo:lo+4, :]
            sl = ug_sb[:, lo:lo + 4, :]
            nc.vector.tensor_reduce(esum[:, lo:lo + 4], sl,
                                    axis=mybir.AxisListType.X,
                                    op=mybir.AluOpType.add)
            nc.vector.reciprocal(rec[:, lo:lo + 4], esum[:, lo:lo + 4])
            for ti in range(lo, lo + 4):
                nc.vector.tensor_scalar_mul(ug_sb[:, ti, :], ug_sb[:, ti, :],
                                            rec[:, ti:ti + 1])

        # exp for ti0-3 (frees lgA's bank for phase B), then normalize so
        # every epilogue uses proper softmax weights (no final rescale).
        nc.scalar.activation(ug_sb[:, 0:4, :], lgA[:, :],
                             mybir.ActivationFunctionType.Exp)
        normalize(0)

        # Phase B: remaining expert-0 groups; gating ti4-7 rides in the
        # (ti4-7, pc0) groups into lgB.
        def b_group(ti, pc, with_gate, lg=None, g_start=False, g_stop=False):
            ps = psum.tile([128, PCHUNK], _F32, name=f"psB{ti}_{pc}", tag="ps")
            for di in range(DT):
                last = di == DT - 1
                expert_mm(ps, w0, ti, pc, di, di == 0, last)
                if with_gate:
                    gate_mm(lg, ti, 4, di,
                            start=(g_start and di == 0),
                            stop=(g_stop and last))
            return ps

        ps_b1 = b_group(3, 1, False)
        # phase-A epilogues (wait on exp of lgA on the DVE queue)
        for ti in range(3):
            for pc in range(PC):
                epilogue(0, ti, pc, ps_a[ti, pc])
        epilogue(0, 3, 0, ps_a[3, 0])
        epilogue(0, 3, 1, ps_b1)

        lgB = None
        ps_pend = []
        for ti in range(4, TT):
            if lgB is None:
                lgB = psum.tile([128, 4 * E], _F32, name="lgB", tag="ps")
                if has_gb:
                    nc.tensor.matmul(lgB[:, :], ones_sb[:1, :], gb4_sb[:1, :],
                                     start=True, stop=False)
            ps = b_group(ti, 0, True, lgB,
                         g_start=(ti == 4 and not has_gb), g_stop=(ti == TT - 1))
            ps_pend.append((ti, 0, ps))
        nc.scalar.activation(ug_sb[:, 4:8, :], lgB[:, :],
                             mybir.ActivationFunctionType.Exp)
        normalize(4)
        for ti, pc, ps in ps_pend:
            epilogue(0, ti, pc, ps)
        for ti in range(4, TT):
            ps = b_group(ti, 1, False)
            epilogue(0, ti, 1, ps)

        if has_eb:
            # ugT[e, t] for the expert-bias mix matmul
            for ti in range(TT):
                ps_t = psum.tile([128, PCHUNK], _F32, name="ps_t", tag="ps")
                gt_ps = ps_t[:E, :128]
                nc.tensor.transpose(gt_ps, ug_sb[:, ti, :], id_sb[:, :])
                nc.scalar.copy(ugt_sb[:, ti * 128:(ti + 1) * 128], gt_ps)

        # ---------------- experts 1..7 ----------------
        for e in range(1, E):
            wt = []
            for di in range(DT):
                w_tile = w_pool.tile([128, P], _BF16, name=f"wt{e}_{di}",
                                     tag="wt")
                nc.sync.dma_start(w_tile[:, :],
                                  ew[e, di * 128:(di + 1) * 128, :])
                wt.append(w_tile)
            for ti in range(TT):
                for pc in range(PC):
                    if e == E - 1 and ti == TT - 1 and pc == PC - 1 \
                            and not has_eb:
                        # Final group split into two N=256 halves: the first
                        # half's store chain (stt + DMA) overlaps the second
                        # half's matmuls, shortening the kernel tail.
                        stg = stage_pool.tile([128, PCHUNK], out_dt,
                                              name="stg")
                        ug_col = ug_sb[:, ti, e:e + 1]
                        for h in range(2):
                            ps = psum.tile([128, PCHUNK], _F32,
                                           name=f"ps{e}_{ti}_{pc}_{h}",
                                           tag="ps")
                            lo = pc * PCHUNK + h * 256
                            for di in range(DT):
                                nc.tensor.matmul(
                                    ps[:, 0:256],
                                    xt[:, di, ti * 128:(ti + 1) * 128],
                                    wt[di][:, lo:lo + 256],
                                    start=(di == 0), stop=(di == DT - 1))
                            nc.vector.scalar_tensor_tensor(
                                stg[:, h * 256:(h + 1) * 256], ps[:, 0:256],
                                ug_col, acc[:, ti, lo:lo + 256],
                                op0=mybir.AluOpType.mult,
                                op1=mybir.AluOpType.add)
                            nc.sync.dma_start(
                                out_t[:, ti, lo:lo + 256],
                                stg[:, h * 256:(h + 1) * 256])
                        continue
                    ps = psum.tile([128, PCHUNK], _F32,
                                   name=f"ps{e}_{ti}_{pc}", tag="ps")
                    for di in range(DT):
                        expert_mm(ps, wt, ti, pc, di, di == 0, di == DT - 1)
                    epilogue(e, ti, pc, ps)

    nc.compile()
    return nc


def _get_module(has_gb: bool, has_eb: bool) -> bass.Bass:
    key = (has_gb, has_eb)
    if key not in _build_cache:
        _build_cache[key] = _build(*key)
    return _build_cache[key]


_last_results = None


def _host_inputs(x, gate_w, gate_b, expert_w, expert_b):
    import ml_dtypes
    bf16 = ml_dtypes.bfloat16

    has_gb = bool(np.any(np.asarray(gate_b)))
    has_eb = bool(np.any(np.asarray(expert_b)))

    x_flat = np.asarray(x, dtype=np.float32).reshape(TOK, D)
    gw = np.asarray(gate_w, np.float32)           # [D, E]
    # gw_p[dp, dt*8+e] = gw[dt*128+dp, e]
    gw_p = np.ascontiguousarray(
        gw.reshape(DT, 128, E).transpose(1, 0, 2).reshape(128, DT * E)
    ).astype(bf16)
    ew_h = np.ascontiguousarray(np.asarray(expert_w, np.float32)).astype(bf16)

    common = {"gw_p": gw_p, "expert_w": ew_h}
    if has_gb:
        gb = np.asarray(gate_b, np.float32).reshape(1, E)
        common["gb4"] = np.tile(gb, (1, 4)).astype(bf16)
        common["ones"] = np.ones((1, 128), dtype=bf16)
    if has_eb:
        common["expert_b"] = np.asarray(expert_b, np.float32).astype(bf16)
        common["ident"] = np.eye(128, dtype=np.float32)

    in_maps = []
    for c in range(N_CORES):
        shard = x_flat[c * TS:(c + 1) * TS]                  # [TS, D]
        xT_h = np.ascontiguousarray(shard.T).astype(bf16)    # [D, TS]
        in_maps.append({"xT": xT_h, **common})
    return in_maps, has_gb, has_eb


def kernel(x, gate_w, gate_b, expert_w, expert_b):
    global _last_results
    in_maps, has_gb, has_eb = _host_inputs(x, gate_w, gate_b,
                                           expert_w, expert_b)
    nc = _get_module(has_gb, has_eb)

    res = run_bass_kernel_spmd(nc, in_maps, core_ids=list(range(N_CORES)),
                               trace=TRACE)
    _last_results = res

    out = np.concatenate([res.results[c]["out"] for c in range(N_CORES)], axis=0)
    return out.reshape(B, T, P).astype(np.float32)


# revision 23
# speedup vs baseline: 1.1986x; 1.1986x over previous
"""MoE layer (dense experts) on 8 Trainium2 NeuronCores via Bass/Tile.

Problem (hardcoded shapes):
  x        [4, 2048, 1024] f32
  gate_w   [1024, 8] f32, gate_b [8] f32
  expert_w [8, 1024, 1024] f32, expert_b [8, 1024] f32
  out[b,t,p] = sum_e softmax(x @ gate_w + gate_b)[b,t,e]
               * (x @ expert_w[e] + expert_b[e])[b,t,p]

Sharding: data-parallel over tokens. 8192 tokens are split into 8 shards of
1024; every core gets the full gate/expert weights (replicated) and computes
its token shard end-to-end. No collectives.

Per-core kernel (x pre-transposed on host so the contraction dim is the
partition dim for both matmul operands). Design notes from trace analysis:
steady state is already at the PE roofline (one N=512 bf16 matmul issued
every ~216ns, LDWEIGHTS hidden by the background weight buffer), so the
whole game is the ramp-in, the gating overhead, and the tail:

  - DMA order: gw first, then (xt_d, w0_d) pairs per contraction tile so
    expert-0 compute starts as soon as the first 512KB lands, then w1..w7
    (paced by the weight pool slots).
  - Warm-up matmuls on a memset tile fill the ~10us preamble+DMA-latency
    window so the PE's HAM clock gate is already at 2.4GHz when real work
    arrives.
  - Phase A (d-outer over 7 psum banks: ti0-2 x both p-halves + ti3 pc0)
    tracks the arriving DMA pairs 1:1. All gating logits for ti0-3 ride in
    one extra psum bank as a single accumulation group (one start=True mm
    clears the bank; later mms overwrite-or-accumulate per has_written),
    each issued right after an expert mm with the same stationary xt tile
    so its LDWEIGHTS hides under the 216ns expert matmul.
  - Phase B: remaining expert-0 groups group-major; gating for ti4-7 rides
    in the (ti4-7, pc0) groups the same way into a second logits bank.
  - Softmax: g = exp(logits) on ACT (no max subtraction: logits are O(6)
    here), then one reduce + reciprocal + 4 tiny scalar-muls per logits
    bank normalizes in place, long before the weights are consumed. The
    epilogue accumulates acc += g_e * psum_e on DVE; the last expert's
    epilogue writes the bf16 store tile directly. This removes all
    per-token-tile PE transposes, the gate-bias matmuls, and the
    expert-bias matmuls (biases are all zero in this problem; nonzero
    biases take a build-time fallback path).
  - Experts 1..7: plain group-major d-loops, epilogue chains on DVE.
"""

import os
from contextlib import ExitStack

import numpy as np

import concourse.bacc as bacc
import concourse.bass as bass
import concourse.mybir as mybir
import concourse.tile as tile
from concourse.bass_utils import run_bass_kernel_spmd

B, T, D, E, P = 4, 2048, 1024, 8, 1024
N_CORES = 8
TOK = B * T                # 8192 tokens
TS = TOK // N_CORES        # 1024 tokens per core
DT = D // 128              # 8 contraction tiles
TT = TS // 128             # 8 token tiles per core
PCHUNK = 512               # psum bank free size (f32)
PC = P // PCHUNK           # 2 p-chunks
WARM = int(os.environ.get("MOE_WARM", "30"))  # HAM warm-up matmuls (N=128)
OUT32 = os.environ.get("MOE_OUT32", "0") == "1"  # f32 output stores

_F32 = mybir.dt.float32
_BF16 = mybir.dt.bfloat16

TRACE = os.environ.get("MOE_TRACE", "0") == "1"  # test.py sets this

_build_cache = {}


def _build(has_gb: bool, has_eb: bool) -> bass.Bass:
    nc = bacc.Bacc("TRN2", target_bir_lowering=False, debug=False,
                   num_devices=N_CORES)

    xT = nc.dram_tensor("xT", [D, TS], _BF16, kind="ExternalInput").ap()
    # gw packed on host: gw_p[dp, dt*8+e] = gate_w[dt*128+dp, e]
    gw = nc.dram_tensor("gw_p", [128, DT * E], _BF16, kind="ExternalInput").ap()
    ew = nc.dram_tensor("expert_w", [E, D, P], _BF16, kind="ExternalInput").ap()
    if has_gb:
        gb4 = nc.dram_tensor("gb4", [1, 4 * E], _BF16, kind="ExternalInput").ap()
        ones = nc.dram_tensor("ones", [1, 128], _BF16, kind="ExternalInput").ap()
    if has_eb:
        eb = nc.dram_tensor("expert_b", [E, P], _BF16, kind="ExternalInput").ap()
        ident = nc.dram_tensor("ident", [128, 128], _F32,
                               kind="ExternalInput").ap()
    out_dt = _F32 if OUT32 else _BF16
    out = nc.dram_tensor("out", [TS, P], out_dt, kind="ExternalOutput").ap()

    out_t = out.rearrange("(tt tp) p -> tp tt p", tp=128)
    xT_t = xT.rearrange("(dt dp) t -> dp dt t", dp=128)

    with tile.TileContext(nc) as tc, ExitStack() as ctx:
        consts = ctx.enter_context(tc.tile_pool(name="consts", bufs=1))
        w_pool = ctx.enter_context(tc.tile_pool(name="w", bufs=32))
        stage_pool = ctx.enter_context(tc.tile_pool(name="stage", bufs=4))
        psum = ctx.enter_context(tc.tile_pool(name="psum", bufs=8, space="PSUM"))

        # --- warm-up: keep PE busy through the preamble + DMA latency so the
        # HAM clock gate is at 2.4GHz when the first real tiles land. The
        # memset rides on GpSimd (whose queue is free right at preamble end,
        # unlike DVE) and is narrow so the first matmul issues ASAP; N=128
        # matmuls give fine-grained filler that ends just before the first
        # (xt, w0) pair lands.
        wu = consts.tile([128, 128], _BF16, name="wu")
        nc.gpsimd.memset(wu[:, :], 1.0)
        wu_ps = psum.tile([128, PCHUNK], _F32, name="wu_ps", tag="ps")
        for _ in range(WARM):
            nc.tensor.matmul(wu_ps[:, :128], wu[:, :], wu[:, :],
                             start=True, stop=True)

        # --- DMA order: first (xt_0, w0_0) pair so expert-0 compute can
        # start ASAP, then gw (needed by the first gating mm, which only
        # issues after the pair-0 expert mms), then the remaining pairs,
        # then the other experts' weights (paced by the pool slots).
        gw_sb = consts.tile([128, DT, E], _BF16, name="gw_sb")
        xt = consts.tile([128, DT, TS], _BF16, name="xt")
        w0 = []
        HTS = TS // 2
        for di in range(DT):
            # Phase A only touches token columns 0..511 (ti0-3): ship that
            # half with w0 so the per-d critical DMA is 384KB, not 512KB —
            # phase A then runs PE-bound with no DMA-jitter stalls. The
            # ti4-7 halves (needed first by phase B) follow after the pairs.
            # Each DMA_DIRECT2D costs ~600ns of serial descriptor processing
            # on its queue, so the ramp-critical pair is split across BOTH
            # hwdge queues (sync + scalar): descriptors process in parallel
            # and the ramp is HBM-bound, not descriptor-bound. The scalar
            # queue carries ONLY these 9 ramp DMAs — its backlog clears by
            # ~13us, long before exp-A needs the engine.
            nc.sync.dma_start(xt[:, di, 0:HTS], xT_t[:, di, 0:HTS])
            w_tile = w_pool.tile([128, P], _BF16, name=f"wt0_{di}", tag="wt")
            nc.scalar.dma_start(w_tile[:, :], ew[0, di * 128:(di + 1) * 128, :])
            w0.append(w_tile)
            if di == 0:
                nc.sync.dma_start(gw_sb[:, :, :],
                                  gw.rearrange("p (dt e) -> p dt e", dt=DT))
                if has_gb:
                    gb4_sb = consts.tile([1, 4 * E], _BF16, name="gb4_sb")
                    nc.sync.dma_start(gb4_sb[:, :], gb4)
                    ones_sb = consts.tile([1, 128], _BF16, name="ones_sb")
                    nc.sync.dma_start(ones_sb[:, :], ones)
                if has_eb:
                    eb_sb = consts.tile([E, P], _BF16, name="eb_sb")
                    nc.sync.dma_start(eb_sb[:, :], eb)
                    id_sb = consts.tile([128, 128], _F32, name="id_sb")
                    nc.sync.dma_start(id_sb[:, :], ident)
        for di in range(DT):
            nc.sync.dma_start(xt[:, di, HTS:TS], xT_t[:, di, HTS:TS])

        ug_sb = consts.tile([128, TT, E], _F32, name="ug_sb")   # exp(logits)
        esum = consts.tile([128, TT], _F32, name="esum")
        rec = consts.tile([128, TT], _F32, name="rec")
        acc = consts.tile([128, TT, P], _F32, name="acc")
        if has_eb:
            ugt_sb = consts.tile([E, TS], _BF16, name="ugt_sb")

        def gate_mm(lg, ti, lo, di, start, stop):
            # logits[t, e] for token tile ti into the packed logits bank;
            # shares its stationary xt tile with the expert mm just issued.
            nc.tensor.matmul(lg[:, (ti - lo) * E:(ti - lo + 1) * E],
                             xt[:, di, ti * 128:(ti + 1) * 128],
                             gw_sb[:, di, :], start=start, stop=stop)

        def expert_mm(ps, e_wt, ti, pc, di, start, stop):
            nc.tensor.matmul(ps[:, :],
                             xt[:, di, ti * 128:(ti + 1) * 128],
                             e_wt[di][:, pc * PCHUNK:(pc + 1) * PCHUNK],
                             start=start, stop=stop)

        def epilogue(e, ti, pc, ps):
            ug_col = ug_sb[:, ti, e:e + 1]
            acc_sl = acc[:, ti, pc * PCHUNK:(pc + 1) * PCHUNK]
            if e == 0:
                nc.vector.tensor_scalar_mul(acc_sl, ps[:, :], ug_col)
            elif e < E - 1:
                nc.vector.scalar_tensor_tensor(
                    acc_sl, ps[:, :], ug_col, acc_sl,
                    op0=mybir.AluOpType.mult, op1=mybir.AluOpType.add)
            else:
                stg = stage_pool.tile([128, PCHUNK], out_dt, name="stg")
                if has_eb:
                    # gate-weighted expert_b mix: gT.T @ eb, K=E
                    ps_b = psum.tile([128, PCHUNK], _F32,
                                     name=f"psb{ti}_{pc}", tag="ps")
                    nc.tensor.matmul(
                        ps_b[:, :], ugt_sb[:, ti * 128:(ti + 1) * 128],
                        eb_sb[:, pc * PCHUNK:(pc + 1) * PCHUNK],
                        start=True, stop=True)
                    tmp = stage_pool.tile([128, PCHUNK], _F32, name="tmp",
                                          tag="tmp")
                    nc.vector.scalar_tensor_tensor(
                        tmp[:, :], ps[:, :], ug_col, acc_sl,
                        op0=mybir.AluOpType.mult, op1=mybir.AluOpType.add)
                    nc.vector.tensor_tensor(
                        stg[:, :], tmp[:, :], ps_b[:, :],
                        op=mybir.AluOpType.add)
                else:
                    nc.vector.scalar_tensor_tensor(
                        stg[:, :], ps[:, :], ug_col, acc_sl,
                        op0=mybir.AluOpType.mult, op1=mybir.AluOpType.add)
                nc.sync.dma_start(
                    out_t[:, ti, pc * PCHUNK:(pc + 1) * PCHUNK], stg[:, :])

        # ---------------- expert 0 + gating ----------------
        # Phase A: d-outer over 7 banks (ti0-2 x pc0/pc1, ti3 pc0) + the
        # ti0-3 logits bank; tracks the (xt_d, w0_d) DMA pairs.
        lgA = psum.tile([128, 4 * E], _F32, name="lgA", tag="ps")
        ps_a = {}
        for ti in range(3):
            for pc in range(PC):
                ps_a[ti, pc] = psum.tile([128, PCHUNK], _F32,
                                         name=f"psA{ti}_{pc}", tag="ps")
        ps_a[3, 0] = psum.tile([128, PCHUNK], _F32, name="psA3_0", tag="ps")

        if has_gb:
            # rank-1 gate-bias preload as lgA's group opener
            nc.tensor.matmul(lgA[:, :], ones_sb[:1, :], gb4_sb[:1, :],
                             start=True, stop=False)
        for di in range(DT):
            last = di == DT - 1
            if last:
                # ti3 first on the last d-tile so lgA's group closes a few
                # matmuls early — its bank (and exp/normalize) then frees
                # right as phase A ends, un-stalling phase B's first group.
                expert_mm(ps_a[3, 0], w0, 3, 0, di, False, True)
                gate_mm(lgA, 3, 0, di, start=False, stop=False)
            for ti in range(3):
                expert_mm(ps_a[ti, 0], w0, ti, 0, di, di == 0, last)
                expert_mm(ps_a[ti, 1], w0, ti, 1, di, di == 0, last)
                gate_mm(lgA, ti, 0, di,
                        start=(di == 0 and ti == 0 and not has_gb),
                        stop=(last and ti == 2))
            if not last:
                expert_mm(ps_a[3, 0], w0, 3, 0, di, di == 0, False)
                gate_mm(lgA, 3, 0, di, start=False, stop=False)

        def normalize(lo):
            # g = exp(logits) / rowsum, in place on ug_sb[:, lo:lo+4, :]
            sl = ug_sb[:, lo:lo + 4, :]
            nc.vector.tensor_reduce(esum[:, lo:lo + 4], sl,
                                    axis=mybir.AxisListType.X,
                                    op=mybir.AluOpType.add)
            nc.vector.reciprocal(rec[:, lo:lo + 4], esum[:, lo:lo + 4])
            for ti in range(lo, lo + 4):
                nc.vector.tensor_scalar_mul(ug_sb[:, ti, :], ug_sb[:, ti, :],
                                            rec[:, ti:ti + 1])

        # exp for ti0-3 (frees lgA's bank for phase B), then normalize so
        # every epilogue uses proper softmax weights (no final rescale).
        nc.scalar.activation(ug_sb[:, 0:4, :], lgA[:, :],
                             mybir.ActivationFunctionType.Exp)
        normalize(0)

        # Phase B: remaining expert-0 groups; gating ti4-7 rides in the
        # (ti4-7, pc0) groups into lgB.
        def b_group(ti, pc, with_gate, lg=None, g_start=False, g_stop=False):
            ps = psum.tile([128, PCHUNK], _F32, name=f"psB{ti}_{pc}", tag="ps")
            for di in range(DT):
                last = di == DT - 1
                expert_mm(ps, w0, ti, pc, di, di == 0, last)
                if with_gate:
                    gate_mm(lg, ti, 4, di,
                            start=(g_start and di == 0),
                            stop=(g_stop and last))
            return ps

        ps_b1 = b_group(3, 1, False)
        # phase-A epilogues (wait on exp of lgA on the DVE queue)
        for ti in range(3):
            for pc in range(PC):
                epilogue(0, ti, pc, ps_a[ti, pc])
        epilogue(0, 3, 0, ps_a[3, 0])
        epilogue(0, 3, 1, ps_b1)

        lgB = None
        ps_pend = []
        for ti in range(4, TT):
            if lgB is None:
                lgB = psum.tile([128, 4 * E], _F32, name="lgB", tag="ps")
                if has_gb:
                    nc.tensor.matmul(lgB[:, :], ones_sb[:1, :], gb4_sb[:1, :],
                                     start=True, stop=False)
            ps = b_group(ti, 0, True, lgB,
                         g_start=(ti == 4 and not has_gb), g_stop=(ti == TT - 1))
            ps_pend.append((ti, 0, ps))
        nc.scalar.activation(ug_sb[:, 4:8, :], lgB[:, :],
                             mybir.ActivationFunctionType.Exp)
        normalize(4)
        for ti, pc, ps in ps_pend:
            epilogue(0, ti, pc, ps)
        for ti in range(4, TT):
            ps = b_group(ti, 1, False)
            epilogue(0, ti, 1, ps)

        if has_eb:
            # ugT[e, t] for the expert-bias mix matmul
            for ti in range(TT):
                ps_t = psum.tile([128, PCHUNK], _F32, name="ps_t", tag="ps")
                gt_ps = ps_t[:E, :128]
                nc.tensor.transpose(gt_ps, ug_sb[:, ti, :], id_sb[:, :])
                nc.scalar.copy(ugt_sb[:, ti * 128:(ti + 1) * 128], gt_ps)

        # ---------------- experts 1..7 ----------------
        for e in range(1, E):
            wt = []
            for di in range(DT):
                w_tile = w_pool.tile([128, P], _BF16, name=f"wt{e}_{di}",
                                     tag="wt")
                nc.sync.dma_start(w_tile[:, :],
                                  ew[e, di * 128:(di + 1) * 128, :])
                wt.append(w_tile)
            for ti in range(TT):
                for pc in range(PC):
                    if e == E - 1 and ti == TT - 1 and pc == PC - 1 \
                            and not has_eb:
                        # Final group split into two N=256 halves: the first
                        # half's store chain (stt + DMA) overlaps the second
                        # half's matmuls, shortening the kernel tail.
                        stg = stage_pool.tile([128, PCHUNK], out_dt,
                                              name="stg")
                        ug_col = ug_sb[:, ti, e:e + 1]
                        for h in range(2):
                            ps = psum.tile([128, PCHUNK], _F32,
                                           name=f"ps{e}_{ti}_{pc}_{h}",
                                           tag="ps")
                            lo = pc * PCHUNK + h * 256
                            for di in range(DT):
                                nc.tensor.matmul(
                                    ps[:, 0:256],
                                    xt[:, di, ti * 128:(ti + 1) * 128],
                                    wt[di][:, lo:lo + 256],
                                    start=(di == 0), stop=(di == DT - 1))
                            nc.vector.scalar_tensor_tensor(
                                stg[:, h * 256:(h + 1) * 256], ps[:, 0:256],
                                ug_col, acc[:, ti, lo:lo + 256],
                                op0=mybir.AluOpType.mult,
                                op1=mybir.AluOpType.add)
                            nc.sync.dma_start(
                                out_t[:, ti, lo:lo + 256],
                                stg[:, h * 256:(h + 1) * 256])
                        continue
                    ps = psum.tile([128, PCHUNK], _F32,
                                   name=f"ps{e}_{ti}_{pc}", tag="ps")
                    for di in range(DT):
                        expert_mm(ps, wt, ti, pc, di, di == 0, di == DT - 1)
                    epilogue(e, ti, pc, ps)

    nc.compile()
    return nc


def _get_module(has_gb: bool, has_eb: bool) -> bass.Bass:
    key = (has_gb, has_eb)
    if key not in _build_cache:
        _build_cache[key] = _build(*key)
    return _build_cache[key]


_last_results = None


def _host_inputs(x, gate_w, gate_b, expert_w, expert_b):
    import ml_dtypes
    bf16 = ml_dtypes.bfloat16

    has_gb = bool(np.any(np.asarray(gate_b)))
    has_eb = bool(np.any(np.asarray(expert_b)))

    x_flat = np.asarray(x, dtype=np.float32).reshape(TOK, D)
    gw = np.asarray(gate_w, np.float32)           # [D, E]
    # gw_p[dp, dt*8+e] = gw[dt*128+dp, e]
    gw_p = np.ascontiguousarray(
        gw.reshape(DT, 128, E).transpose(1, 0, 2).reshape(128, DT * E)
    ).astype(bf16)
    ew_h = np.ascontiguousarray(np.asarray(expert_w, np.float32)).astype(bf16)

    common = {"gw_p": gw_p, "expert_w": ew_h}
    if has_gb:
        gb = np.asarray(gate_b, np.float32).reshape(1, E)
        common["gb4"] = np.tile(gb, (1, 4)).astype(bf16)
        common["ones"] = np.ones((1, 128), dtype=bf16)
    if has_eb:
        common["expert_b"] = np.asarray(expert_b, np.float32).astype(bf16)
        common["ident"] = np.eye(128, dtype=np.float32)

    in_maps = []
    for c in range(N_CORES):
        shard = x_flat[c * TS:(c + 1) * TS]                  # [TS, D]
        xT_h = np.ascontiguousarray(shard.T).astype(bf16)    # [D, TS]
        in_maps.append({"xT": xT_h, **common})
    return in_maps, has_gb, has_eb


def kernel(x, gate_w, gate_b, expert_w, expert_b):
    global _last_results
    in_maps, has_gb, has_eb = _host_inputs(x, gate_w, gate_b,
                                           expert_w, expert_b)
    nc = _get_module(has_gb, has_eb)

    res = run_bass_kernel_spmd(nc, in_maps, core_ids=list(range(N_CORES)),
                               trace=TRACE)
    _last_results = res

    out = np.concatenate([res.results[c]["out"] for c in range(N_CORES)], axis=0)
    return out.reshape(B, T, P).astype(np.float32)
